# revision 1
# baseline (speedup 1.0000x reference)
"""Complex self-attention on 8 Trainium2 NeuronCores (Bass/Tile).

Model (reference): complex linear q/k/v projections of (x_re, x_im), attention
scores = (Re(q)·Re(k) + Im(q)·Im(k))/sqrt(D), softmax, attn applied to Re(v)
and Im(v), complex output projection. B=2, N=2048, C=1024, H=16, D=64.

Sharding: heads tensor-parallel across 8 cores (2 heads/core, both batches).
  - Projections use a stacked-contraction trick: [x_re; x_im] (2C=2048 rows)
    against host-prebuilt combined weights, so each complex part is ONE matmul
    chain. q/k produce [ch, tokens] layout; v produces [tokens, ch].
  - Scores are computed transposed (keys on partitions) so exp-scores feed the
    attn@v matmul directly. The softmax denominator is accumulated on the PE
    (ones[128,128] @ exp-tile, PSUM-accumulated over key tiles) which keeps the
    tensor engine dense (HAM-warm) and the vector engine light. No
    max-subtraction (scores/8 within ±6 for this input distribution).
  - The v-projection bias is folded into the output-projection bias on the
    host (softmax rows sum to 1, so it adds a constant per channel).
  - Attention outputs are exchanged with an AllToAll (4 MB/core) so each core
    finishes the complex output projection for a 512-token slice with full
    channel visibility — no AllReduce needed.
All matmuls run as float32r (full PE rate at free-dim >= 256, ~tf32 accuracy).
"""

import sys

if "/opt/trn_rl_repo" not in sys.path:
    sys.path.insert(0, "/opt/trn_rl_repo")

from contextlib import ExitStack

import numpy as np

import concourse.mybir as mybir
import concourse.tile as tile
from concourse import bacc
from concourse.bass_utils import run_bass_kernel_spmd

B, N, C = 2, 2048, 1024
H, D = 16, 64
T = B * N  # 4096 tokens total
NCORES = 8
HPC = H // NCORES  # 2 heads per core
TSL = T // NCORES  # 512-token output slice per core
TF = 512  # projection token-chunk (free dim)
KT = 2 * C // 128  # 16 contraction tiles of 128 over [x_re; x_im]
F32 = mybir.dt.float32
F32R = mybir.dt.float32r


def _host_prep(inp):
    """Build the host-side sharded/combined arrays (see numpy_model.py)."""
    x_re = np.ascontiguousarray(np.asarray(inp["x_re"], dtype=np.float32).reshape(T, C))
    x_im = np.ascontiguousarray(np.asarray(inp["x_im"], dtype=np.float32).reshape(T, C))
    xT2 = np.ascontiguousarray(np.concatenate([x_re.T, x_im.T], axis=0))  # [2C, T]

    per_core = []
    for c in range(NCORES):
        d = {}
        h0 = c * HPC
        ch = slice(h0 * D, (h0 + HPC) * D)
        for nm in ("q", "k", "v"):
            Wre = np.asarray(inp[f"{nm}_Wre"], dtype=np.float32)[ch]  # [128, C]
            Wim = np.asarray(inp[f"{nm}_Wim"], dtype=np.float32)[ch]
            bre = np.asarray(inp[f"{nm}_bre"], dtype=np.float32)[ch]
            bim = np.asarray(inp[f"{nm}_bim"], dtype=np.float32)[ch]
            Ws, bs = [], []
            for hh in range(HPC):
                hs = slice(hh * D, (hh + 1) * D)
                wr = np.concatenate([Wre[hs].T, -Wim[hs].T], axis=0)  # [2C, 64]
                wi = np.concatenate([Wim[hs].T, Wre[hs].T], axis=0)
                Ws.append(np.concatenate([wr, wi], axis=1))  # [2C, 128]
                bs.append(np.concatenate([bre[hs] - bim[hs], bre[hs] + bim[hs]]))
            if nm == "v":
                d["wv"] = np.ascontiguousarray(np.concatenate(Ws, axis=1))  # [2C, 256]
            else:
                d[f"w{nm}"] = np.ascontiguousarray(np.stack(Ws))  # [HPC, 2C, 128]
                d[f"b{nm}"] = np.ascontiguousarray(np.stack(bs, axis=1))  # [128, HPC]
        per_core.append(d)

    # o-projection combined matrices, rows ordered to match the A2A result:
    # rank r, then per rank [h0:out_r(64), h0:out_i(64), h1:out_r(64), h1:out_i(64)]
    oWre = np.asarray(inp["o_Wre"], dtype=np.float32)
    oWim = np.asarray(inp["o_Wim"], dtype=np.float32)
    Mre_rows, Mim_rows = [], []
    bv_rows = []
    for r in range(NCORES):
        vbre = np.asarray(inp["v_bre"], dtype=np.float32)
        vbim = np.asarray(inp["v_bim"], dtype=np.float32)
        for hh in range(HPC):
            h = r * HPC + hh
            hs = slice(h * D, (h + 1) * D)
            Mre_rows += [oWre[:, hs].T, -oWim[:, hs].T]
            Mim_rows += [oWim[:, hs].T, oWre[:, hs].T]
            bv_rows += [vbre[hs] - vbim[hs], vbre[hs] + vbim[hs]]
    M_re = np.ascontiguousarray(np.concatenate(Mre_rows, axis=0))  # [2C, C]
    M_im = np.ascontiguousarray(np.concatenate(Mim_rows, axis=0))
    bv_full = np.concatenate(bv_rows)  # [2C] — v bias in A2A row order
    o_bre = np.asarray(inp["o_bre"], dtype=np.float32)
    o_bim = np.asarray(inp["o_bim"], dtype=np.float32)
    # fold the v bias through the o-projection (softmax rows sum to 1)
    bo_re = (o_bre - o_bim) + M_re.T @ bv_full  # [C]
    bo_im = (o_bre + o_bim) + M_im.T @ bv_full
    bo_re = np.ascontiguousarray(bo_re.reshape(8, 128).T.astype(np.float32))  # [128, 8]
    bo_im = np.ascontiguousarray(bo_im.reshape(8, 128).T.astype(np.float32))
    shared = dict(xT2=xT2, M_re=M_re, M_im=M_im, bo_re=bo_re, bo_im=bo_im)
    return shared, per_core


def _build_program():
    nc = bacc.Bacc("TRN2", target_bir_lowering=False, debug=False, num_devices=NCORES)

    # ---- DRAM I/O ----
    xT2_d = nc.dram_tensor("xT2", [2 * C, T], F32, kind="ExternalInput")
    wq_d = nc.dram_tensor("wq", [HPC, 2 * C, 128], F32, kind="ExternalInput")
    wk_d = nc.dram_tensor("wk", [HPC, 2 * C, 128], F32, kind="ExternalInput")
    wv_d = nc.dram_tensor("wv", [2 * C, 2 * HPC * D], F32, kind="ExternalInput")
    bq_d = nc.dram_tensor("bq", [128, HPC], F32, kind="ExternalInput")
    bk_d = nc.dram_tensor("bk", [128, HPC], F32, kind="ExternalInput")
    Mre_d = nc.dram_tensor("M_re", [2 * C, C], F32, kind="ExternalInput")
    Mim_d = nc.dram_tensor("M_im", [2 * C, C], F32, kind="ExternalInput")
    bore_d = nc.dram_tensor("bo_re", [128, 8], F32, kind="ExternalInput")
    boim_d = nc.dram_tensor("bo_im", [128, 8], F32, kind="ExternalInput")
    yout_d = nc.dram_tensor("yout", [2 * C, TSL], F32, kind="ExternalOutput")

    xT2_t = xT2_d.rearrange("(kt p) t -> p kt t", p=128)  # [128, 16, T]
    NCH = N // TF  # chunks per batch

    with (
        tile.TileContext(nc) as tc,
        nc.allow_low_precision(
            reason="fp32r intermediates; rounding noise matches fp32r matmul noise"
        ),
    ):
        with tc.tile_pool(name="dram", bufs=1, space="DRAM") as dram:
            outc_dram = dram.tile([NCORES, HPC * 128, TSL], F32R)
            at_dram = dram.tile([NCORES, HPC * 128, TSL], F32R)

            # pool stack is LIFO per memory space: open in reverse close order —
            # keep (last close), then attention-phase (ctx2), then proj-phase
            # (ctx1, closed first, right after the b1 projections)
            with tc.tile_pool(name="keep", bufs=1) as keep:
                ctx2 = ExitStack()  # attention-phase pools: freed before the o-phase
                qk_sb = ctx2.enter_context(tc.tile_pool(name="qk_sb", bufs=1))
                v_sbp = ctx2.enter_context(tc.tile_pool(name="v_sb", bufs=1))
                expp = ctx2.enter_context(tc.tile_pool(name="expp", bufs=3))
                evp = ctx2.enter_context(tc.tile_pool(name="evp", bufs=3))
                sc_ps = ctx2.enter_context(tc.tile_pool(name="sc_ps", bufs=2, space="PSUM"))
                av_ps = ctx2.enter_context(tc.tile_pool(name="av_ps", bufs=2, space="PSUM"))
                den_ps = ctx2.enter_context(
                    tc.tile_pool(name="den_ps", bufs=1, space="PSUM")
                )

                ctx1 = ExitStack()  # proj-phase pools: freed before the o-phase
                const = ctx1.enter_context(tc.tile_pool(name="const", bufs=1))
                xp = ctx1.enter_context(tc.tile_pool(name="xp", bufs=2))
                qk_ps = ctx1.enter_context(
                    tc.tile_pool(name="qk_ps", bufs=2, space="PSUM")
                )
                v_ps = ctx1.enter_context(tc.tile_pool(name="v_ps", bufs=1, space="PSUM"))
                # ---- first x chunk DMA goes out before the weights ----
                xt0 = xp.tile([128, KT, TF], F32R, name="xt", tag="xt")
                nc.sync.dma_start(xt0[:], xT2_t[:, :, 0:TF].bitcast(F32R))

                # ---- constants (weights on the gpsimd DMA queues) ----
                wq_sb = const.tile([128, HPC, KT, 128], F32R)
                wk_sb = const.tile([128, HPC, KT, 128], F32R)
                wv_sb = const.tile([128, KT, 2 * HPC * D], F32R)
                nc.gpsimd.dma_start(
                    wq_sb[:], wq_d.rearrange("h (kt p) m -> p h kt m", p=128).bitcast(F32R)
                )
                nc.gpsimd.dma_start(
                    wk_sb[:], wk_d.rearrange("h (kt p) m -> p h kt m", p=128).bitcast(F32R)
                )
                nc.gpsimd.dma_start(
                    wv_sb[:], wv_d.rearrange("(kt p) m -> p kt m", p=128).bitcast(F32R)
                )
                bq_sb = keep.tile([128, HPC], F32)
                bk_sb = keep.tile([128, HPC], F32)
                nc.gpsimd.dma_start(bq_sb[:], bq_d[:])
                nc.gpsimd.dma_start(bk_sb[:], bk_d[:])
                ones_f = keep.tile([128, 128], F32)
                nc.any.memset(ones_f[:], 1.0)
                ones128 = keep.tile([128, 128], F32R)
                nc.vector.tensor_copy(ones128[:], ones_f[:])

                for b in range(B):
                    # ---- projections for this batch's 2048 tokens ----
                    qc = {}
                    kc = {}
                    vt = {}
                    for hh in range(HPC):
                        qc[hh] = qk_sb.tile([128, N], F32R, name=f"qc{hh}", tag=f"qc{hh}")
                        kc[hh] = qk_sb.tile([128, N], F32R, name=f"kc{hh}", tag=f"kc{hh}")
                        vt[hh] = v_sbp.tile(
                            [128, N // 128, 128], F32R, name=f"vt{hh}", tag=f"vt{hh}"
                        )
                    for ci in range(NCH):
                        t0 = b * N + ci * TF
                        if b == 0 and ci == 0:
                            xt = xt0
                        else:
                            xt = xp.tile([128, KT, TF], F32R, name="xt", tag="xt")
                            nc.sync.dma_start(xt[:], xT2_t[:, :, t0 : t0 + TF].bitcast(F32R))
                        csl = slice(ci * TF, ci * TF + TF)
                        for hh in range(HPC):
                            for which, w_sb, b_sb, dst in (
                                ("q", wq_sb, bq_sb, qc[hh]),
                                ("k", wk_sb, bk_sb, kc[hh]),
                            ):
                                ps = qk_ps.tile([128, TF], F32, name="qkps", tag="qkps")
                                for kt in range(KT):
                                    nc.tensor.matmul(
                                        ps[:],
                                        w_sb[:, hh, kt, :],
                                        xt[:, kt, :],
                                        start=(kt == 0),
                                        stop=(kt == KT - 1),
                                    )
                                nc.scalar.activation(
                                    dst[:, csl],
                                    ps[:],
                                    mybir.ActivationFunctionType.Identity,
                                    bias=b_sb[:, hh : hh + 1],
                                )
                        for m in range(TF // 128):
                            vp = v_ps.tile([128, 2 * HPC * D], F32, name="vps", tag="vps")
                            for kt in range(KT):
                                nc.tensor.matmul(
                                    vp[:],
                                    xt[:, kt, m * 128 : (m + 1) * 128],
                                    wv_sb[:, kt, :],
                                    start=(kt == 0),
                                    stop=(kt == KT - 1),
                                )
                            ktok = ci * (TF // 128) + m
                            for hh in range(HPC):
                                nc.vector.tensor_copy(
                                    vt[hh][:, ktok, :], vp[:, hh * 128 : (hh + 1) * 128]
                                )

                    # ---- attention ----
                    for hh in range(HPC):
                        for qt in range(N // 512):
                            qsl = slice(qt * 512, (qt + 1) * 512)
                            av = av_ps.tile([128, 512], F32, name="avps", tag="avps")
                            den = den_ps.tile([128, 512], F32, name="denps", tag="denps")
                            for kt in range(N // 128):
                                sc = sc_ps.tile([128, 512], F32, name="scps", tag="scps")
                                nc.tensor.matmul(
                                    sc[:],
                                    kc[hh][:, kt * 128 : (kt + 1) * 128],
                                    qc[hh][:, qsl],
                                    start=True,
                                    stop=True,
                                )
                                ex = expp.tile([128, 512], F32R, name="ex", tag="ex")
                                nc.scalar.activation(
                                    ex[:], sc[:], mybir.ActivationFunctionType.Exp, scale=0.125
                                )
                                nc.tensor.matmul(
                                    av[:],
                                    vt[hh][:, kt, :],
                                    ex[:],
                                    start=(kt == 0),
                                    stop=(kt == N // 128 - 1),
                                )
                                nc.tensor.matmul(
                                    den[:],
                                    ones128[:],
                                    ex[:],
                                    start=(kt == 0),
                                    stop=(kt == N // 128 - 1),
                                )
                            rb_sb = evp.tile([128, 512], F32, name="rb_sb", tag="rb_sb")
                            nc.vector.reciprocal(rb_sb[:], den[:])
                            outc = evp.tile([128, 512], F32R, name="outc", tag="outc")
                            nc.vector.tensor_tensor(
                                outc[:], av[:], rb_sb[:], mybir.AluOpType.mult
                            )
                            j = b * (N // 512) + qt
                            nc.sync.dma_start(
                                outc_dram[j, hh * 128 : (hh + 1) * 128, :], outc[:]
                            )

                    if b == B - 1:
                        # free proj-phase SBUF/PSUM so the o-phase loads below can
                        # start (their deps: last b1-projection reads) while b1
                        # attention still runs
                        ctx1.close()

                # attention emission done; free its pools so the o-phase loads
                # reuse the space (address-reuse deps let the M loads start as
                # soon as the overlapped ranges' last readers finish)
                ctx2.close()

                # ---- o-phase loads (overlap b1 attention + the AllToAll) ----
                with (
                    tc.tile_pool(name="oc", bufs=1) as oc,
                    tc.tile_pool(name="mp", bufs=4) as mp,
                    tc.tile_pool(name="oev", bufs=3) as oev,
                    tc.tile_pool(name="o_ps", bufs=2, space="PSUM") as o_ps,
                ):
                    m_tiles = []
                    for g in range(2):
                        for part, M_d in ((0, Mre_d), (1, Mim_d)):
                            m_sb = mp.tile(
                                [128, KT, 512], F32R, name=f"m_sb{g}{part}", tag="m_sb"
                            )
                            nc.sync.dma_start(
                                m_sb[:],
                                M_d.rearrange("(kt p) o -> p kt o", p=128)[
                                    :, :, g * 512 : (g + 1) * 512
                                ].bitcast(F32R),
                            )
                            m_tiles.append((g, part, m_sb))
                    bore_sb = oc.tile([128, 8], F32)
                    boim_sb = oc.tile([128, 8], F32)
                    nc.gpsimd.dma_start(bore_sb[:], bore_d[:])
                    nc.gpsimd.dma_start(boim_sb[:], boim_d[:])

                    # ---- exchange attention outputs ----
                    nc.gpsimd.collective_compute(
                        "AllToAll",
                        mybir.AluOpType.bypass,
                        replica_groups=[list(range(NCORES))],
                        ins=[outc_dram.opt()],
                        outs=[at_dram.opt()],
                    )

                    # ---- output projection for this core's 512-token slice ----
                    at_t = at_dram.rearrange("r (hp p) t -> p (r hp) t", p=128)
                    at_sb = oc.tile([128, KT, TSL], F32R)
                    nc.sync.dma_start(at_sb[:], at_t[:, :, :])
                    for g, part, m_sb in m_tiles:
                        bo_sb = bore_sb if part == 0 else boim_sb
                        for i in range(4):
                            ocht = g * 4 + i
                            ps = o_ps.tile([128, TSL], F32, name="ops", tag="ops")
                            for kt in range(KT):
                                nc.tensor.matmul(
                                    ps[:],
                                    m_sb[:, kt, i * 128 : (i + 1) * 128],
                                    at_sb[:, kt, :],
                                    start=(kt == 0),
                                    stop=(kt == KT - 1),
                                )
                            y_sb = oev.tile([128, TSL], F32, name="y_sb", tag="y_sb")
                            nc.scalar.activation(
                                y_sb[:],
                                ps[:],
                                mybir.ActivationFunctionType.Identity,
                                bias=bo_sb[:, ocht : ocht + 1],
                            )
                            nc.sync.dma_start(
                                yout_d[part * C + ocht * 128 : part * C + (ocht + 1) * 128, :],
                                y_sb[:],
                            )
    nc.compile()
    return nc


_NC_CACHE = None


def _get_program():
    global _NC_CACHE
    if _NC_CACHE is None:
        _NC_CACHE = _build_program()
    return _NC_CACHE


def _run(inputs, trace=False, trace_kwargs=None):
    shared, per_core = _host_prep(inputs)
    nc = _get_program()
    in_maps = []
    for c in range(NCORES):
        d = per_core[c]
        in_maps.append(
            {
                "xT2": shared["xT2"],
                "wq": d["wq"],
                "wk": d["wk"],
                "wv": d["wv"],
                "bq": d["bq"],
                "bk": d["bk"],
                "M_re": shared["M_re"],
                "M_im": shared["M_im"],
                "bo_re": shared["bo_re"],
                "bo_im": shared["bo_im"],
            }
        )
    res = run_bass_kernel_spmd(
        nc, in_maps, list(range(NCORES)), trace=trace, **(trace_kwargs or {})
    )
    youts = [res.results[c]["yout"] for c in range(NCORES)]
    re = np.concatenate([y[:C] for y in youts], axis=1)  # [C, T]
    im = np.concatenate([y[C:] for y in youts], axis=1)
    out = np.stack(
        [re.T.reshape(B, N, C), im.T.reshape(B, N, C)]
    ).astype(np.float32)
    return out, res


def kernel(**inputs) -> np.ndarray:
    out, _ = _run(inputs, trace=False)
    return out



# revision 3
# speedup vs baseline: 1.1611x; 1.1611x over previous
"""Complex self-attention on 8 Trainium2 NeuronCores (Bass/Tile), v2.

Model (reference): complex linear q/k/v projections of (x_re, x_im), attention
scores = (Re(q)·Re(k) + Im(q)·Im(k))/sqrt(D), softmax, attn applied to Re(v)
and Im(v), complex output projection. B=2, N=2048, C=1024, H=16, D=64.

Sharding: heads tensor-parallel across 8 cores (2 heads/core, both batches).

v2 changes over the fp32r baseline (724µs):
  - bf16 everywhere (weights, x, q/k/v, exp-scores, attention outputs, o-proj
    matrices). PE rate is identical to fp32r (1 cycle/row) but DMA bytes halve,
    FWL (fast weight load) activates, and SBUF pressure halves. PSUM
    accumulation stays fp32.
  - Host pre-permutes every DRAM tensor into partition-major layout so each
    DMA is 128 large contiguous descriptors (the baseline's 512B-chunk gathers
    made the first x+weight load a 55µs stall).
  - All projections (both batches) run first; attention follows with the full
    8 PSUM banks: scores for a PAIR of key-tiles accumulate into one
    [128,1024] PSUM tile so a single Exp activation covers 1024 elements
    (halves ACT instruction overhead; scalar was pacing the attention loop).
  - den (softmax denominator) and av PSUM pools are double-buffered so the
    slow DVE reciprocal is off the PE critical path.
  - The AllToAll is split per batch ([8,256,256] bf16, ~1MB/core each):
    A2A(b0) overlaps batch-1 attention; A2A(b1) overlaps the b0 output
    projection. The baseline's single 4MB fp32 A2A exposed a 108µs stall.
"""

import sys

if "/opt/trn_rl_repo" not in sys.path:
    sys.path.insert(0, "/opt/trn_rl_repo")

from contextlib import ExitStack

import ml_dtypes
import numpy as np

import concourse.mybir as mybir
import concourse.tile as tile
from concourse import bacc
from concourse.bass_utils import run_bass_kernel_spmd

B, N, C = 2, 2048, 1024
H, D = 16, 64
T = B * N  # 4096 tokens total
NCORES = 8
HPC = H // NCORES  # 2 heads per core
TF = 512  # projection token-chunk (free dim)
NCHT = T // TF  # 8 chunks total over both batches
KT = 2 * C // 128  # 16 contraction tiles of 128 over [x_re; x_im]
TOKB = 256  # A2A token block (per batch, per core slice)
TSL = 512  # final per-core output token count (256 from each batch)
F32 = mybir.dt.float32
BF16 = mybir.dt.bfloat16
BF = ml_dtypes.bfloat16


def _host_prep(inp):
    """Pre-permute all tensors to partition-major bf16 for contiguous DMA."""
    f32 = np.float32
    x_re = np.asarray(inp["x_re"], f32).reshape(T, C)
    x_im = np.asarray(inp["x_im"], f32).reshape(T, C)
    xT2 = np.concatenate([x_re.T, x_im.T], axis=0)  # [2C, T]
    # [2C, T] -> [chunk, p, kt, t] so each 512-token chunk is one contiguous
    # 16KB block per partition
    xc = np.ascontiguousarray(
        xT2.reshape(KT, 128, NCHT, TF).transpose(2, 1, 0, 3).astype(BF)
    )

    per_core = []
    for c in range(NCORES):
        d = {}
        h0 = c * HPC
        ch = slice(h0 * D, (h0 + HPC) * D)
        for nm in ("q", "k", "v"):
            Wre = np.asarray(inp[f"{nm}_Wre"], f32)[ch]  # [128, C]
            Wim = np.asarray(inp[f"{nm}_Wim"], f32)[ch]
            bre = np.asarray(inp[f"{nm}_bre"], f32)[ch]
            bim = np.asarray(inp[f"{nm}_bim"], f32)[ch]
            Ws, bs = [], []
            for hh in range(HPC):
                hs = slice(hh * D, (hh + 1) * D)
                wr = np.concatenate([Wre[hs].T, -Wim[hs].T], axis=0)  # [2C, 64]
                wi = np.concatenate([Wim[hs].T, Wre[hs].T], axis=0)
                Ws.append(np.concatenate([wr, wi], axis=1))  # [2C, 128]
                bs.append(np.concatenate([bre[hs] - bim[hs], bre[hs] + bim[hs]]))
            if nm == "v":
                wv = np.concatenate(Ws, axis=1)  # [2C, 256]
                d["wv"] = np.ascontiguousarray(
                    wv.reshape(KT, 128, 2 * HPC * D).transpose(1, 0, 2).astype(BF)
                )  # [128, KT, 256]
            else:
                w = np.stack(Ws)  # [HPC, 2C, 128]
                d[f"w{nm}"] = np.ascontiguousarray(
                    w.reshape(HPC, KT, 128, 128).transpose(2, 0, 1, 3).astype(BF)
                )  # [128, HPC, KT, 128]
                d[f"b{nm}"] = np.ascontiguousarray(np.stack(bs, axis=1))  # [128, HPC]
        per_core.append(d)

    # o-projection combined matrices, rows ordered to match the A2A result:
    # rank r, then per rank [h0:out_r(64), h0:out_i(64), h1:out_r(64), h1:out_i(64)]
    oWre = np.asarray(inp["o_Wre"], f32)
    oWim = np.asarray(inp["o_Wim"], f32)
    vbre = np.asarray(inp["v_bre"], f32)
    vbim = np.asarray(inp["v_bim"], f32)
    Mre_rows, Mim_rows, bv_rows = [], [], []
    for r in range(NCORES):
        for hh in range(HPC):
            h = r * HPC + hh
            hs = slice(h * D, (h + 1) * D)
            Mre_rows += [oWre[:, hs].T, -oWim[:, hs].T]
            Mim_rows += [oWim[:, hs].T, oWre[:, hs].T]
            bv_rows += [vbre[hs] - vbim[hs], vbre[hs] + vbim[hs]]
    M_re = np.concatenate(Mre_rows, axis=0)  # [2C, C]
    M_im = np.concatenate(Mim_rows, axis=0)
    bv_full = np.concatenate(bv_rows)  # [2C] — v bias in A2A row order
    o_bre = np.asarray(inp["o_bre"], f32)
    o_bim = np.asarray(inp["o_bim"], f32)
    # fold the v bias through the o-projection (softmax rows sum to 1)
    bo_re = (o_bre - o_bim) + M_re.T @ bv_full  # [C]
    bo_im = (o_bre + o_bim) + M_im.T @ bv_full
    bo_re = np.ascontiguousarray(bo_re.reshape(8, 128).T.astype(f32))  # [128, 8]
    bo_im = np.ascontiguousarray(bo_im.reshape(8, 128).T.astype(f32))
    m_re = np.ascontiguousarray(
        M_re.reshape(KT, 128, C).transpose(1, 0, 2).astype(BF)
    )  # [128, KT, C]
    m_im = np.ascontiguousarray(M_im.reshape(KT, 128, C).transpose(1, 0, 2).astype(BF))
    shared = dict(xc=xc, m_re=m_re, m_im=m_im, bo_re=bo_re, bo_im=bo_im)
    return shared, per_core


def _build_program():
    nc = bacc.Bacc("TRN2", target_bir_lowering=False, debug=False, num_devices=NCORES)

    # ---- DRAM I/O (all partition-major, contiguous per partition) ----
    xc_d = nc.dram_tensor("xc", [NCHT, 128, KT, TF], BF16, kind="ExternalInput")
    wq_d = nc.dram_tensor("wq", [128, HPC, KT, 128], BF16, kind="ExternalInput")
    wk_d = nc.dram_tensor("wk", [128, HPC, KT, 128], BF16, kind="ExternalInput")
    wv_d = nc.dram_tensor("wv", [128, KT, 2 * HPC * D], BF16, kind="ExternalInput")
    bq_d = nc.dram_tensor("bq", [128, HPC], F32, kind="ExternalInput")
    bk_d = nc.dram_tensor("bk", [128, HPC], F32, kind="ExternalInput")
    mre_d = nc.dram_tensor("m_re", [128, KT, C], BF16, kind="ExternalInput")
    mim_d = nc.dram_tensor("m_im", [128, KT, C], BF16, kind="ExternalInput")
    bore_d = nc.dram_tensor("bo_re", [128, 8], F32, kind="ExternalInput")
    boim_d = nc.dram_tensor("bo_im", [128, 8], F32, kind="ExternalInput")
    yout_d = nc.dram_tensor("yout", [2 * C, TSL], F32, kind="ExternalOutput")

    NCH = N // TF  # chunks per batch

    with (
        tile.TileContext(nc) as tc,
        nc.allow_low_precision(reason="bf16 compute; fp32 PSUM accumulation"),
    ):
        with tc.tile_pool(name="dram", bufs=1, space="DRAM") as dram:
            out_d = [
                dram.tile([NCORES, HPC * 128, TOKB], BF16, name=f"out_d{b}")
                for b in range(B)
            ]
            at_d = [
                dram.tile([NCORES, HPC * 128, TOKB], BF16, name=f"at_d{b}")
                for b in range(B)
            ]

            with tc.tile_pool(name="keep", bufs=1) as keep:
                # qc/kc/vt for both batches stay live through attention
                qc, kc, vt = {}, {}, {}
                with tc.tile_pool(name="qckc", bufs=1) as qckc:
                    for b in range(B):
                        for hh in range(HPC):
                            qc[b, hh] = qckc.tile(
                                [128, N], BF16, name=f"qc{b}{hh}", tag=f"qc{b}{hh}"
                            )
                            kc[b, hh] = qckc.tile(
                                [128, N], BF16, name=f"kc{b}{hh}", tag=f"kc{b}{hh}"
                            )
                            vt[b, hh] = qckc.tile(
                                [128, N // 128, 128],
                                BF16,
                                name=f"vt{b}{hh}",
                                tag=f"vt{b}{hh}",
                            )

                    ctxP = ExitStack()  # projection-phase pools
                    const = ctxP.enter_context(tc.tile_pool(name="const", bufs=1))
                    xp = ctxP.enter_context(tc.tile_pool(name="xp", bufs=2))
                    qk_ps = ctxP.enter_context(
                        tc.tile_pool(name="qk_ps", bufs=2, space="PSUM")
                    )
                    v_ps = ctxP.enter_context(
                        tc.tile_pool(name="v_ps", bufs=2, space="PSUM")
                    )

                    # ---- first x chunk DMA goes out before the weights ----
                    xt0 = xp.tile([128, KT, TF], BF16, name="xt", tag="xt")
                    nc.sync.dma_start(xt0[:], xc_d[0])

                    wq_sb = const.tile([128, HPC, KT, 128], BF16)
                    wk_sb = const.tile([128, HPC, KT, 128], BF16)
                    wv_sb = const.tile([128, KT, 2 * HPC * D], BF16)
                    nc.gpsimd.dma_start(wq_sb[:], wq_d[:])
                    nc.gpsimd.dma_start(wk_sb[:], wk_d[:])
                    nc.gpsimd.dma_start(wv_sb[:], wv_d[:])
                    bq_sb = keep.tile([128, HPC], F32)
                    bk_sb = keep.tile([128, HPC], F32)
                    nc.gpsimd.dma_start(bq_sb[:], bq_d[:])
                    nc.gpsimd.dma_start(bk_sb[:], bk_d[:])
                    bore_sb = keep.tile([128, 8], F32)
                    boim_sb = keep.tile([128, 8], F32)
                    nc.gpsimd.dma_start(bore_sb[:], bore_d[:])
                    nc.gpsimd.dma_start(boim_sb[:], boim_d[:])
                    ones_f = keep.tile([128, 128], F32)
                    nc.any.memset(ones_f[:], 1.0)
                    ones128 = keep.tile([128, 128], BF16)
                    nc.vector.tensor_copy(ones128[:], ones_f[:])

                    # ---- projections: both batches ----
                    for chunk in range(NCHT):
                        b, ci = divmod(chunk, NCH)
                        if chunk == 0:
                            xt = xt0
                        else:
                            xt = xp.tile([128, KT, TF], BF16, name="xt", tag="xt")
                            nc.sync.dma_start(xt[:], xc_d[chunk])
                        csl = slice(ci * TF, ci * TF + TF)
                        for hh in range(HPC):
                            for w_sb, b_sb, dst in (
                                (wq_sb, bq_sb, qc[b, hh]),
                                (wk_sb, bk_sb, kc[b, hh]),
                            ):
                                ps = qk_ps.tile([128, TF], F32, name="qkps", tag="qkps")
                                for kt in range(KT):
                                    nc.tensor.matmul(
                                        ps[:],
                                        w_sb[:, hh, kt, :],
                                        xt[:, kt, :],
                                        start=(kt == 0),
                                        stop=(kt == KT - 1),
                                    )
                                nc.scalar.activation(
                                    dst[:, csl],
                                    ps[:],
                                    mybir.ActivationFunctionType.Identity,
                                    bias=b_sb[:, hh : hh + 1],
                                )
                        for m in range(TF // 128):
                            vp = v_ps.tile(
                                [128, 2 * HPC * D], F32, name="vps", tag="vps"
                            )
                            for kt in range(KT):
                                nc.tensor.matmul(
                                    vp[:],
                                    xt[:, kt, m * 128 : (m + 1) * 128],
                                    wv_sb[:, kt, :],
                                    start=(kt == 0),
                                    stop=(kt == KT - 1),
                                )
                            ktok = ci * (TF // 128) + m
                            for hh in range(HPC):
                                nc.vector.tensor_copy(
                                    vt[b, hh][:, ktok, :],
                                    vp[:, hh * 128 : (hh + 1) * 128],
                                )
                    ctxP.close()

                    # ---- o-projection loads (overlap attention) ----
                    ctxM = ExitStack()
                    mp = ctxM.enter_context(tc.tile_pool(name="mp", bufs=1))
                    mre_sb = mp.tile([128, KT, C], BF16, name="mre", tag="mre")
                    mim_sb = mp.tile([128, KT, C], BF16, name="mim", tag="mim")
                    nc.gpsimd.dma_start(mre_sb[:], mre_d[:])
                    nc.gpsimd.dma_start(mim_sb[:], mim_d[:])

                    # ---- attention (full 8 PSUM banks available) ----
                    ctxA = ExitStack()
                    expp = ctxA.enter_context(tc.tile_pool(name="expp", bufs=3))
                    evp = ctxA.enter_context(tc.tile_pool(name="evp", bufs=3))
                    sc_ps = ctxA.enter_context(
                        tc.tile_pool(name="sc_ps", bufs=2, space="PSUM")
                    )
                    av_ps = ctxA.enter_context(
                        tc.tile_pool(name="av_ps", bufs=2, space="PSUM")
                    )
                    den_ps = ctxA.enter_context(
                        tc.tile_pool(name="den_ps", bufs=2, space="PSUM")
                    )
                    NKP = N // 256  # key-tile pairs
                    for b in range(B):
                        for hh in range(HPC):
                            for qt in range(N // 512):
                                qsl = slice(qt * 512, qt * 512 + 512)
                                av = av_ps.tile([128, 512], F32, name="avps", tag="avps")
                                den = den_ps.tile(
                                    [128, 512], F32, name="denps", tag="denps"
                                )
                                for ktp in range(NKP):
                                    sc = sc_ps.tile(
                                        [128, 1024], F32, name="scps", tag="scps"
                                    )
                                    for half in range(2):
                                        kt = ktp * 2 + half
                                        nc.tensor.matmul(
                                            sc[:, half * 512 : half * 512 + 512],
                                            kc[b, hh][:, kt * 128 : (kt + 1) * 128],
                                            qc[b, hh][:, qsl],
                                            start=True,
                                            stop=True,
                                        )
                                    ex = expp.tile([128, 1024], BF16, name="ex", tag="ex")
                                    nc.scalar.activation(
                                        ex[:],
                                        sc[:],
                                        mybir.ActivationFunctionType.Exp,
                                        scale=0.125,
                                    )
                                    for half in range(2):
                                        kt = ktp * 2 + half
                                        first = ktp == 0 and half == 0
                                        last = ktp == NKP - 1 and half == 1
                                        exh = ex[:, half * 512 : half * 512 + 512]
                                        nc.tensor.matmul(
                                            av[:],
                                            vt[b, hh][:, kt, :],
                                            exh,
                                            start=first,
                                            stop=last,
                                        )
                                        nc.tensor.matmul(
                                            den[:],
                                            ones128[:],
                                            exh,
                                            start=first,
                                            stop=last,
                                        )
                                rb = evp.tile([128, 512], F32, name="rb", tag="rb")
                                nc.vector.reciprocal(rb[:], den[:])
                                outc = evp.tile([128, 512], BF16, name="outc", tag="outc")
                                nc.vector.tensor_tensor(
                                    outc[:], av[:], rb[:], mybir.AluOpType.mult
                                )
                                for s in range(2):
                                    nc.sync.dma_start(
                                        out_d[b][
                                            qt * 2 + s, hh * 128 : (hh + 1) * 128, :
                                        ],
                                        outc[:, s * 256 : s * 256 + 256],
                                    )
                        # exchange this batch's attention outputs (overlaps the
                        # next batch's attention / the previous o-projection)
                        nc.gpsimd.collective_compute(
                            "AllToAll",
                            mybir.AluOpType.bypass,
                            replica_groups=[list(range(NCORES))],
                            ins=[out_d[b].opt()],
                            outs=[at_d[b].opt()],
                        )
                    ctxA.close()

                    # ---- output projection: 256-token slice per batch ----
                    with (
                        tc.tile_pool(name="oc", bufs=1) as oc,
                        tc.tile_pool(name="oev", bufs=3) as oev,
                        tc.tile_pool(name="o_ps", bufs=4, space="PSUM") as o_ps,
                    ):
                        for b in range(B):
                            at_sb = oc.tile(
                                [128, KT, TOKB], BF16, name=f"at{b}", tag=f"at{b}"
                            )
                            nc.sync.dma_start(
                                at_sb[:],
                                at_d[b].rearrange("r (hp p) t -> p (r hp) t", p=128),
                            )
                            for part, (m_sb, bo_sb) in enumerate(
                                ((mre_sb, bore_sb), (mim_sb, boim_sb))
                            ):
                                for ot in range(C // 128):
                                    ps = o_ps.tile(
                                        [128, TOKB], F32, name="ops", tag="ops"
                                    )
                                    for kt in range(KT):
                                        nc.tensor.matmul(
                                            ps[:],
                                            m_sb[:, kt, ot * 128 : (ot + 1) * 128],
                                            at_sb[:, kt, :],
                                            start=(kt == 0),
                                            stop=(kt == KT - 1),
                                        )
                                    y_sb = oev.tile(
                                        [128, TOKB], F32, name="y_sb", tag="y_sb"
                                    )
                                    nc.scalar.activation(
                                        y_sb[:],
                                        ps[:],
                                        mybir.ActivationFunctionType.Identity,
                                        bias=bo_sb[:, ot : ot + 1],
                                    )
                                    nc.sync.dma_start(
                                        yout_d[
                                            part * C + ot * 128 : part * C
                                            + (ot + 1) * 128,
                                            b * TOKB : (b + 1) * TOKB,
                                        ],
                                        y_sb[:],
                                    )
                    ctxM.close()
    nc.compile()
    return nc


_NC_CACHE = None


def _get_program():
    global _NC_CACHE
    if _NC_CACHE is None:
        _NC_CACHE = _build_program()
    return _NC_CACHE


def _run(inputs, trace=False, trace_kwargs=None):
    shared, per_core = _host_prep(inputs)
    nc = _get_program()
    in_maps = []
    for c in range(NCORES):
        d = per_core[c]
        in_maps.append(
            {
                "xc": shared["xc"],
                "wq": d["wq"],
                "wk": d["wk"],
                "wv": d["wv"],
                "bq": d["bq"],
                "bk": d["bk"],
                "m_re": shared["m_re"],
                "m_im": shared["m_im"],
                "bo_re": shared["bo_re"],
                "bo_im": shared["bo_im"],
            }
        )
    res = run_bass_kernel_spmd(
        nc, in_maps, list(range(NCORES)), trace=trace, **(trace_kwargs or {})
    )
    youts = [res.results[c]["yout"] for c in range(NCORES)]
    re = np.empty((C, B, N), dtype=np.float32)
    im = np.empty((C, B, N), dtype=np.float32)
    for c in range(NCORES):
        y = youts[c]
        tsl = slice(c * TOKB, (c + 1) * TOKB)
        re[:, 0, tsl] = y[:C, :TOKB]
        re[:, 1, tsl] = y[:C, TOKB:]
        im[:, 0, tsl] = y[C:, :TOKB]
        im[:, 1, tsl] = y[C:, TOKB:]
    out = np.stack([re.transpose(1, 2, 0), im.transpose(1, 2, 0)]).astype(np.float32)
    return out, res


def kernel(**inputs) -> np.ndarray:
    out, _ = _run(inputs, trace=False)
    return out


# revision 7
# speedup vs baseline: 1.3954x; 1.2017x over previous
"""Complex self-attention on 8 Trainium2 NeuronCores (Bass/Tile), v2.

Model (reference): complex linear q/k/v projections of (x_re, x_im), attention
scores = (Re(q)·Re(k) + Im(q)·Im(k))/sqrt(D), softmax, attn applied to Re(v)
and Im(v), complex output projection. B=2, N=2048, C=1024, H=16, D=64.

Sharding: heads tensor-parallel across 8 cores (2 heads/core, both batches).

v2 changes over the fp32r baseline (724µs):
  - bf16 everywhere (weights, x, q/k/v, exp-scores, attention outputs, o-proj
    matrices). PE rate is identical to fp32r (1 cycle/row) but DMA bytes halve,
    FWL (fast weight load) activates, and SBUF pressure halves. PSUM
    accumulation stays fp32.
  - Host pre-permutes every DRAM tensor into partition-major layout so each
    DMA is 128 large contiguous descriptors (the baseline's 512B-chunk gathers
    made the first x+weight load a 55µs stall).
  - All projections (both batches) run first; attention follows with the full
    8 PSUM banks: scores for a PAIR of key-tiles accumulate into one
    [128,1024] PSUM tile so a single Exp activation covers 1024 elements
    (halves ACT instruction overhead; scalar was pacing the attention loop).
  - den (softmax denominator) and av PSUM pools are double-buffered so the
    slow DVE reciprocal is off the PE critical path.
  - The AllToAll is split per batch ([8,256,256] bf16, ~1MB/core each):
    A2A(b0) overlaps batch-1 attention; A2A(b1) overlaps the b0 output
    projection. The baseline's single 4MB fp32 A2A exposed a 108µs stall.
"""

import sys

if "/opt/trn_rl_repo" not in sys.path:
    sys.path.insert(0, "/opt/trn_rl_repo")

from contextlib import ExitStack

import ml_dtypes
import numpy as np

import concourse.mybir as mybir
import concourse.tile as tile
from concourse import bacc
from concourse.bass_utils import run_bass_kernel_spmd

B, N, C = 2, 2048, 1024
H, D = 16, 64
T = B * N  # 4096 tokens total
NCORES = 8
HPC = H // NCORES  # 2 heads per core
TF = 512  # projection token-chunk (free dim)
NCHT = T // TF  # 8 chunks total over both batches
KT = 2 * C // 128  # 16 contraction tiles of 128 over [x_re; x_im]
TOKB = 256  # A2A token block (per batch, per core slice)
TSL = 512  # final per-core output token count (256 from each batch)
F32 = mybir.dt.float32
BF16 = mybir.dt.bfloat16
BF = ml_dtypes.bfloat16


def _host_prep(inp):
    """Pre-permute all tensors to partition-major bf16 for contiguous DMA."""
    f32 = np.float32
    x_re = np.asarray(inp["x_re"], f32).reshape(T, C)
    x_im = np.asarray(inp["x_im"], f32).reshape(T, C)
    xT2 = np.concatenate([x_re.T, x_im.T], axis=0)  # [2C, T]
    # [2C, T] -> [chunk, p, kt, t] so each 512-token chunk is one contiguous
    # 16KB block per partition
    xc = np.ascontiguousarray(
        xT2.reshape(KT, 128, NCHT, TF).transpose(2, 1, 0, 3).astype(BF)
    )

    per_core = []
    for c in range(NCORES):
        d = {}
        h0 = c * HPC
        ch = slice(h0 * D, (h0 + HPC) * D)
        for nm in ("q", "k", "v"):
            Wre = np.asarray(inp[f"{nm}_Wre"], f32)[ch]  # [128, C]
            Wim = np.asarray(inp[f"{nm}_Wim"], f32)[ch]
            bre = np.asarray(inp[f"{nm}_bre"], f32)[ch]
            bim = np.asarray(inp[f"{nm}_bim"], f32)[ch]
            Ws, bs = [], []
            for hh in range(HPC):
                hs = slice(hh * D, (hh + 1) * D)
                wr = np.concatenate([Wre[hs].T, -Wim[hs].T], axis=0)  # [2C, 64]
                wi = np.concatenate([Wim[hs].T, Wre[hs].T], axis=0)
                Ws.append(np.concatenate([wr, wi], axis=1))  # [2C, 128]
                bs.append(np.concatenate([bre[hs] - bim[hs], bre[hs] + bim[hs]]))
            if nm == "v":
                wv = np.concatenate(Ws, axis=1)  # [2C, 256]
                d["wv"] = np.ascontiguousarray(
                    wv.reshape(KT, 128, 2 * HPC * D).transpose(1, 0, 2).astype(BF)
                )  # [128, KT, 256]
            else:
                w = np.stack(Ws)  # [HPC, 2C, 128]
                d[f"w{nm}"] = np.ascontiguousarray(
                    w.reshape(HPC, KT, 128, 128).transpose(2, 0, 1, 3).astype(BF)
                )  # [128, HPC, KT, 128]
                d[f"b{nm}"] = np.ascontiguousarray(np.stack(bs, axis=1))  # [128, HPC]
        per_core.append(d)

    # o-projection combined matrices, rows ordered to match the A2A result:
    # rank r, then per rank [h0:out_r(64), h0:out_i(64), h1:out_r(64), h1:out_i(64)]
    oWre = np.asarray(inp["o_Wre"], f32)
    oWim = np.asarray(inp["o_Wim"], f32)
    vbre = np.asarray(inp["v_bre"], f32)
    vbim = np.asarray(inp["v_bim"], f32)
    Mre_rows, Mim_rows, bv_rows = [], [], []
    for r in range(NCORES):
        for hh in range(HPC):
            h = r * HPC + hh
            hs = slice(h * D, (h + 1) * D)
            Mre_rows += [oWre[:, hs].T, -oWim[:, hs].T]
            Mim_rows += [oWim[:, hs].T, oWre[:, hs].T]
            bv_rows += [vbre[hs] - vbim[hs], vbre[hs] + vbim[hs]]
    M_re = np.concatenate(Mre_rows, axis=0)  # [2C, C]
    M_im = np.concatenate(Mim_rows, axis=0)
    bv_full = np.concatenate(bv_rows)  # [2C] — v bias in A2A row order
    o_bre = np.asarray(inp["o_bre"], f32)
    o_bim = np.asarray(inp["o_bim"], f32)
    # fold the v bias through the o-projection (softmax rows sum to 1)
    bo_re = (o_bre - o_bim) + M_re.T @ bv_full  # [C]
    bo_im = (o_bre + o_bim) + M_im.T @ bv_full
    bo_re = np.ascontiguousarray(bo_re.reshape(8, 128).T.astype(f32))  # [128, 8]
    bo_im = np.ascontiguousarray(bo_im.reshape(8, 128).T.astype(f32))
    m_re = np.ascontiguousarray(
        M_re.reshape(KT, 128, C).transpose(1, 0, 2).astype(BF)
    )  # [128, KT, C]
    m_im = np.ascontiguousarray(M_im.reshape(KT, 128, C).transpose(1, 0, 2).astype(BF))
    shared = dict(xc=xc, m_re=m_re, m_im=m_im, bo_re=bo_re, bo_im=bo_im)
    return shared, per_core


def _build_program():
    nc = bacc.Bacc("TRN2", target_bir_lowering=False, debug=False, num_devices=NCORES)

    # ---- DRAM I/O (all partition-major, contiguous per partition) ----
    xc_d = nc.dram_tensor("xc", [NCHT, 128, KT, TF], BF16, kind="ExternalInput")
    wq_d = nc.dram_tensor("wq", [128, HPC, KT, 128], BF16, kind="ExternalInput")
    wk_d = nc.dram_tensor("wk", [128, HPC, KT, 128], BF16, kind="ExternalInput")
    wv_d = nc.dram_tensor("wv", [128, KT, 2 * HPC * D], BF16, kind="ExternalInput")
    bq_d = nc.dram_tensor("bq", [128, HPC], F32, kind="ExternalInput")
    bk_d = nc.dram_tensor("bk", [128, HPC], F32, kind="ExternalInput")
    mre_d = nc.dram_tensor("m_re", [128, KT, C], BF16, kind="ExternalInput")
    mim_d = nc.dram_tensor("m_im", [128, KT, C], BF16, kind="ExternalInput")
    bore_d = nc.dram_tensor("bo_re", [128, 8], F32, kind="ExternalInput")
    boim_d = nc.dram_tensor("bo_im", [128, 8], F32, kind="ExternalInput")
    yout_d = nc.dram_tensor("yout", [2 * C, TSL], F32, kind="ExternalOutput")

    NCH = N // TF  # chunks per batch

    with (
        tile.TileContext(nc) as tc,
        nc.allow_low_precision(reason="bf16 compute; fp32 PSUM accumulation"),
    ):
        with tc.tile_pool(name="dram", bufs=1, space="DRAM") as dram:
            out_d = [
                dram.tile([NCORES, HPC * 128, TOKB], BF16, name=f"out_d{b}")
                for b in range(B)
            ]
            at_d = [
                dram.tile([NCORES, HPC * 128, TOKB], BF16, name=f"at_d{b}")
                for b in range(B)
            ]

            with tc.tile_pool(name="keep", bufs=1) as keep:
                # qc/kc/vt for both batches stay live through attention
                qc, kc, vt = {}, {}, {}
                with tc.tile_pool(name="qckc", bufs=1) as qckc:
                    for b in range(B):
                        for hh in range(HPC):
                            qc[b, hh] = qckc.tile(
                                [128, N], BF16, name=f"qc{b}{hh}", tag=f"qc{b}{hh}"
                            )
                            kc[b, hh] = qckc.tile(
                                [128, N], BF16, name=f"kc{b}{hh}", tag=f"kc{b}{hh}"
                            )
                            vt[b, hh] = qckc.tile(
                                [128, N // 128, 128],
                                BF16,
                                name=f"vt{b}{hh}",
                                tag=f"vt{b}{hh}",
                            )

                    ctxP = ExitStack()  # projection-phase pools
                    const = ctxP.enter_context(tc.tile_pool(name="const", bufs=1))
                    xp = ctxP.enter_context(tc.tile_pool(name="xp", bufs=2))
                    qk_ps = ctxP.enter_context(
                        tc.tile_pool(name="qk_ps", bufs=2, space="PSUM")
                    )
                    v_ps = ctxP.enter_context(
                        tc.tile_pool(name="v_ps", bufs=2, space="PSUM")
                    )

                    # ---- first x chunk DMA goes out before the weights ----
                    xt0 = xp.tile([128, KT, TF], BF16, name="xt", tag="xt")
                    nc.sync.dma_start(xt0[:], xc_d[0])

                    # spread the startup loads over four engine queues so the
                    # first matmul chain (xt0 + wq) isn't serialized behind
                    # everything else
                    wq_sb = const.tile([128, HPC, KT, 128], BF16)
                    wk_sb = const.tile([128, HPC, KT, 128], BF16)
                    wv_sb = const.tile([128, KT, 2 * HPC * D], BF16)
                    nc.gpsimd.dma_start(wq_sb[:], wq_d[:])
                    nc.scalar.dma_start(wk_sb[:], wk_d[:])
                    nc.sync.dma_start(wv_sb[:], wv_d[:])
                    bq_sb = keep.tile([128, HPC], F32)
                    bk_sb = keep.tile([128, HPC], F32)
                    nc.gpsimd.dma_start(bq_sb[:], bq_d[:])
                    nc.gpsimd.dma_start(bk_sb[:], bk_d[:])
                    bore_sb = keep.tile([128, 8], F32)
                    boim_sb = keep.tile([128, 8], F32)
                    nc.gpsimd.dma_start(bore_sb[:], bore_d[:])
                    nc.gpsimd.dma_start(boim_sb[:], boim_d[:])
                    ones_f = keep.tile([128, 128], F32)
                    nc.any.memset(ones_f[:], 1.0)
                    ones128 = keep.tile([128, 128], BF16)
                    nc.vector.tensor_copy(ones128[:], ones_f[:])

                    # ---- projections: both batches ----
                    for chunk in range(NCHT):
                        b, ci = divmod(chunk, NCH)
                        if chunk == 0:
                            xt = xt0
                        else:
                            xt = xp.tile([128, KT, TF], BF16, name="xt", tag="xt")
                            nc.sync.dma_start(xt[:], xc_d[chunk])
                        csl = slice(ci * TF, ci * TF + TF)
                        for hh in range(HPC):
                            for w_sb, b_sb, dst in (
                                (wq_sb, bq_sb, qc[b, hh]),
                                (wk_sb, bk_sb, kc[b, hh]),
                            ):
                                ps = qk_ps.tile([128, TF], F32, name="qkps", tag="qkps")
                                for kt in range(KT):
                                    nc.tensor.matmul(
                                        ps[:],
                                        w_sb[:, hh, kt, :],
                                        xt[:, kt, :],
                                        start=(kt == 0),
                                        stop=(kt == KT - 1),
                                    )
                                nc.scalar.activation(
                                    dst[:, csl],
                                    ps[:],
                                    mybir.ActivationFunctionType.Identity,
                                    bias=b_sb[:, hh : hh + 1],
                                )
                        for m in range(TF // 128):
                            vp = v_ps.tile(
                                [128, 2 * HPC * D], F32, name="vps", tag="vps"
                            )
                            for kt in range(KT):
                                nc.tensor.matmul(
                                    vp[:],
                                    xt[:, kt, m * 128 : (m + 1) * 128],
                                    wv_sb[:, kt, :],
                                    start=(kt == 0),
                                    stop=(kt == KT - 1),
                                )
                            ktok = ci * (TF // 128) + m
                            for hh in range(HPC):
                                nc.vector.tensor_copy(
                                    vt[b, hh][:, ktok, :],
                                    vp[:, hh * 128 : (hh + 1) * 128],
                                )
                    ctxP.close()

                    # ---- o-projection loads (overlap attention) ----
                    ctxM = ExitStack()
                    mp = ctxM.enter_context(tc.tile_pool(name="mp", bufs=1))
                    mre_sb = mp.tile([128, KT, C], BF16, name="mre", tag="mre")
                    mim_sb = mp.tile([128, KT, C], BF16, name="mim", tag="mim")
                    nc.gpsimd.dma_start(mre_sb[:], mre_d[:])
                    nc.gpsimd.dma_start(mim_sb[:], mim_d[:])

                    # at_sb tiles live here so their loads can be issued on the
                    # gpsimd queue right after each collective completes
                    oc = ctxM.enter_context(tc.tile_pool(name="oc", bufs=1))
                    at_sb = {}
                    for b in range(B):
                        at_sb[b] = oc.tile(
                            [128, KT, TOKB], BF16, name=f"at{b}", tag=f"at{b}"
                        )

                    # ---- attention (full 8 PSUM banks available) ----
                    # software-pipelined: the next key-tile-pair's score
                    # matmuls are emitted BEFORE this pair's av/den matmuls so
                    # the (in-order) PE streams through Exp latency.
                    ctxA = ExitStack()
                    expp = ctxA.enter_context(tc.tile_pool(name="expp", bufs=3))
                    evp = ctxA.enter_context(tc.tile_pool(name="evp", bufs=3))
                    sc_ps = ctxA.enter_context(
                        tc.tile_pool(name="sc_ps", bufs=2, space="PSUM")
                    )
                    av_ps = ctxA.enter_context(
                        tc.tile_pool(name="av_ps", bufs=2, space="PSUM")
                    )
                    den_ps = ctxA.enter_context(
                        tc.tile_pool(name="den_ps", bufs=2, space="PSUM")
                    )
                    NKP = N // 256  # key-tile pairs
                    units = [
                        (b, hh, qt)
                        for b in range(B)
                        for hh in range(HPC)
                        for qt in range(N // 512)
                    ]
                    jobs = [(ui, ktp) for ui in range(len(units)) for ktp in range(NKP)]

                    def emit_sc(job):
                        ui, ktp = job
                        b, hh, qt = units[ui]
                        qsl = slice(qt * 512, qt * 512 + 512)
                        sc = sc_ps.tile([128, 1024], F32, name="scps", tag="scps")
                        for half in range(2):
                            kt = ktp * 2 + half
                            nc.tensor.matmul(
                                sc[:, half * 512 : half * 512 + 512],
                                kc[b, hh][:, kt * 128 : (kt + 1) * 128],
                                qc[b, hh][:, qsl],
                                start=True,
                                stop=True,
                            )
                        ex = expp.tile([128, 1024], BF16, name="ex", tag="ex")
                        nc.scalar.activation(
                            ex[:], sc[:], mybir.ActivationFunctionType.Exp, scale=0.125
                        )
                        return ex

                    pend = {0: emit_sc(jobs[0])}
                    acc = {}
                    for j, (ui, ktp) in enumerate(jobs):
                        if j + 1 < len(jobs):
                            pend[j + 1] = emit_sc(jobs[j + 1])
                        b, hh, qt = units[ui]
                        ex = pend.pop(j)
                        if ktp == 0:
                            acc[ui] = (
                                av_ps.tile([128, 512], F32, name="avps", tag="avps"),
                                den_ps.tile([128, 512], F32, name="denps", tag="denps"),
                            )
                        av, den = acc[ui]
                        for half in range(2):
                            kt = ktp * 2 + half
                            first = ktp == 0 and half == 0
                            last = ktp == NKP - 1 and half == 1
                            exh = ex[:, half * 512 : half * 512 + 512]
                            nc.tensor.matmul(
                                av[:], vt[b, hh][:, kt, :], exh, start=first, stop=last
                            )
                            nc.tensor.matmul(
                                den[:], ones128[:], exh, start=first, stop=last
                            )
                        if ktp == NKP - 1:
                            del acc[ui]
                            rb = evp.tile([128, 512], F32, name="rb", tag="rb")
                            nc.vector.reciprocal(rb[:], den[:])
                            outc = evp.tile([128, 512], BF16, name="outc", tag="outc")
                            nc.vector.tensor_tensor(
                                outc[:], av[:], rb[:], mybir.AluOpType.mult
                            )
                            for s in range(2):
                                nc.sync.dma_start(
                                    out_d[b][qt * 2 + s, hh * 128 : (hh + 1) * 128, :],
                                    outc[:, s * 256 : s * 256 + 256],
                                )
                            if (ui + 1) % (len(units) // B) == 0:
                                # exchange this batch's attention outputs
                                # (overlaps the next batch's attention / the
                                # previous o-projection), then pull the result
                                # into SBUF on the same (idle) gpsimd queue
                                nc.gpsimd.collective_compute(
                                    "AllToAll",
                                    mybir.AluOpType.bypass,
                                    replica_groups=[list(range(NCORES))],
                                    ins=[out_d[b].opt()],
                                    outs=[at_d[b].opt()],
                                )
                                nc.gpsimd.dma_start(
                                    at_sb[b][:],
                                    at_d[b].rearrange(
                                        "r (hp p) t -> p (r hp) t", p=128
                                    ),
                                )
                    ctxA.close()

                    # ---- output projection: 256-token slice per batch ----
                    with (
                        tc.tile_pool(name="oev", bufs=3) as oev,
                        tc.tile_pool(name="o_ps", bufs=4, space="PSUM") as o_ps,
                    ):
                        for b in range(B):
                            for part, (m_sb, bo_sb) in enumerate(
                                ((mre_sb, bore_sb), (mim_sb, boim_sb))
                            ):
                                for ot in range(C // 128):
                                    ps = o_ps.tile(
                                        [128, TOKB], F32, name="ops", tag="ops"
                                    )
                                    for kt in range(KT):
                                        nc.tensor.matmul(
                                            ps[:],
                                            m_sb[:, kt, ot * 128 : (ot + 1) * 128],
                                            at_sb[b][:, kt, :],
                                            start=(kt == 0),
                                            stop=(kt == KT - 1),
                                        )
                                    y_sb = oev.tile(
                                        [128, TOKB], F32, name="y_sb", tag="y_sb"
                                    )
                                    nc.scalar.activation(
                                        y_sb[:],
                                        ps[:],
                                        mybir.ActivationFunctionType.Identity,
                                        bias=bo_sb[:, ot : ot + 1],
                                    )
                                    nc.sync.dma_start(
                                        yout_d[
                                            part * C + ot * 128 : part * C
                                            + (ot + 1) * 128,
                                            b * TOKB : (b + 1) * TOKB,
                                        ],
                                        y_sb[:],
                                    )
                    ctxM.close()
    nc.compile()
    return nc


_NC_CACHE = None


def _get_program():
    global _NC_CACHE
    if _NC_CACHE is None:
        _NC_CACHE = _build_program()
    return _NC_CACHE


def _run(inputs, trace=False, trace_kwargs=None):
    shared, per_core = _host_prep(inputs)
    nc = _get_program()
    in_maps = []
    for c in range(NCORES):
        d = per_core[c]
        in_maps.append(
            {
                "xc": shared["xc"],
                "wq": d["wq"],
                "wk": d["wk"],
                "wv": d["wv"],
                "bq": d["bq"],
                "bk": d["bk"],
                "m_re": shared["m_re"],
                "m_im": shared["m_im"],
                "bo_re": shared["bo_re"],
                "bo_im": shared["bo_im"],
            }
        )
    res = run_bass_kernel_spmd(
        nc, in_maps, list(range(NCORES)), trace=trace, **(trace_kwargs or {})
    )
    youts = [res.results[c]["yout"] for c in range(NCORES)]
    re = np.empty((C, B, N), dtype=np.float32)
    im = np.empty((C, B, N), dtype=np.float32)
    for c in range(NCORES):
        y = youts[c]
        tsl = slice(c * TOKB, (c + 1) * TOKB)
        re[:, 0, tsl] = y[:C, :TOKB]
        re[:, 1, tsl] = y[:C, TOKB:]
        im[:, 0, tsl] = y[C:, :TOKB]
        im[:, 1, tsl] = y[C:, TOKB:]
    out = np.stack([re.transpose(1, 2, 0), im.transpose(1, 2, 0)]).astype(np.float32)
    return out, res


def kernel(**inputs) -> np.ndarray:
    out, _ = _run(inputs, trace=False)
    return out
